# revision 11
# baseline (speedup 1.0000x reference)
"""BatchedTreeForest (moe_routing) Trainium2 kernel.

Reference computation (B=4, S=2048, D=1024, O=512, T=16 trees, depth 4):
  logits  = einsum('bsd,tnd->bstn', x, W_dec) + b_dec          (15 internal nodes)
  dec     = sigmoid(logits / softplus(temp_logits + .5413))
  leafp   = prod over the 4 root->leaf path levels of (dec | 1-dec)
  per_tree= einsum('bstl,tlo->bsto', leafp, leaf_outputs)
  gate    = softmax(x @ gate_w + gate_b)
  out     = LayerNorm(einsum('bsto,bst->bso', per_tree, gate)) * gamma + beta

Mapping onto 8 NeuronCores: data-parallel over the 8192 tokens (1024/core),
tree/gate parameters replicated.  Per core, 8 tiles of 128 tokens:
  mm1: xt(k-tiles) x [W_dec | gate_w] -> psum [128 tok, 256 cols]
  ACT sigmoid twice (scale=+1/-1) -> d2 [128, 2, 256]: contiguous [dec|gate]
       blocks for both signs (gate sigmoid/1-sigmoid come for free)
  gate softmax without exp: e = s/(1-s); Z folded into the LN epsilon
  leaf products: gate folded at the root level (32-col op), then a 3-level
       cascade (64/128/256 cols) split DVE/GPSIMD
  PE transpose -> (t,l)-major + two N=1 matmuls against the host-precomputed
       rowsum(leaf)/O column -> per-token mean lands in the same PSUM bank
  mm2 against leaf_outputs; sum(y^2) via one ACT Square w/ accum_out
  LayerNorm: rsqrt(var + eps*Z^2) via magic seed + Newton on DVE, batched
       over (3,3,2) tiles; final (x-mu)*rstd fused, alternating ACT/DVE,
       reading the mm2 PSUM banks directly; stores in bf16 (upcast on host).

Host-side prep re-lays x out so every x DMA moves 4KiB-contiguous partition
lines (128 descriptors/tile instead of 1024).

Matmul inputs run as float32r (reduced-precision fp32 at 4x the fp32 matmul
throughput); set TREE_MM_DTYPE=f32 for exact-fp32 matmuls.  TREE_OUT_DTYPE
(bf16|f32) controls the store dtype.
"""
import os
import sys

sys.path.insert(0, "/opt/trn_rl_repo")

import numpy as np

P = 128
D = 1024
T = 16
NI = 15
NL = 16
NDEC = T * NI  # 240
COLS = NDEC + T  # 256: decision logits | gate logits
O = 512
NCORES = 8
TOK_PC = 1024  # tokens per core
NTILES = TOK_PC // P
KT = D // P  # 8 contraction tiles
EPS = 1e-5
MAGIC = 0x5F3759DF

MM_DTYPE = os.environ.get("TREE_MM_DTYPE", "f32r")
OUT_DTYPE = os.environ.get("TREE_OUT_DTYPE", "bf16")
BATCHES = (3, 3, 2)


def build(mm_dtype: str = MM_DTYPE, apply_affine: bool = False, has_bias: bool = True,
          out_dtype: str = OUT_DTYPE):
    """Build the per-core Bass module.  Returns the Bacc object (uncompiled)."""
    import concourse.bacc as bacc
    import concourse.mybir as mybir
    from concourse import masks
    from concourse.tile import TileContext

    f32 = mybir.dt.float32
    i32 = mybir.dt.int32
    DT = mybir.dt.float32r if mm_dtype == "f32r" else f32
    ODT = mybir.dt.bfloat16 if out_dtype == "bf16" else f32
    Alu = mybir.AluOpType
    Act = mybir.ActivationFunctionType
    X = mybir.AxisListType.X

    nc = bacc.Bacc()
    # Matmul-feeding parameters are declared float32r directly (same bytes as
    # fp32 on the wire -- dt.np(float32r) is np.float32, so the host arrays
    # pass the dtype check).  This keeps every load on HWDGE with no SWDGE
    # cast-DMAs, and satisfies the BIR verifier's fp32r producer rule.
    xT_d = nc.declare_dram_parameter("xT", [NTILES * P, KT * P], DT, isOutput=False)
    wcat_d = nc.declare_dram_parameter("wcatT", [D, COLS], DT, isOutput=False)
    if has_bias:
        bias_d = nc.declare_dram_parameter("biascat", [2, COLS], DT, isOutput=False)
    leaf_d = nc.declare_dram_parameter("leaf2", [T * NL, O + 2], DT, isOutput=False)
    if apply_affine:
        gamma_d = nc.declare_dram_parameter("gamma", [1, O], f32, isOutput=False)
        beta_d = nc.declare_dram_parameter("beta", [1, O], f32, isOutput=False)
    out_d = nc.declare_dram_parameter("out", [TOK_PC, O], ODT, isOutput=True)

    ldeng = nc.sync

    with TileContext(nc) as tc:
        with (
            tc.tile_pool(name="consts", bufs=1) as consts,
            tc.tile_pool(name="xin", bufs=4) as xin,
            tc.tile_pool(name="d2p", bufs=3) as d2p,
            tc.tile_pool(name="mid", bufs=3) as mid,
            tc.tile_pool(name="wtp", bufs=3) as wtp,
            tc.tile_pool(name="scrapp", bufs=2) as scrapp,
            tc.tile_pool(name="outp", bufs=NTILES) as outp,
            tc.tile_pool(name="gatep", bufs=4) as gatep,
            tc.tile_pool(name="statp", bufs=3) as statp,
            tc.tile_pool(name="smalls", bufs=4) as smalls,
            tc.tile_pool(name="psum1", bufs=2, space="PSUM") as psum1,
            tc.tile_pool(name="psumT", bufs=2, space="PSUM") as psumT,
            tc.tile_pool(name="psum3", bufs=4, space="PSUM") as psum3,
        ):
            # ---- constants + first x tiles, ordered so tile 0's matmuls can
            # start as soon as wcat k0 + xt0 have landed (HWDGE drains the SP
            # ring in issue order) ----
            wcat_sb = consts.tile([P, KT, COLS], DT)
            wcat_r = wcat_d[:, :].rearrange("(a p) c -> p a c", p=P)
            ldeng.dma_start(out=wcat_sb[:, 0], in_=wcat_r[:, 0])
            xt0 = xin.tile([P, KT, P], DT, name="xt")
            ldeng.dma_start(out=xt0, in_=xT_d[0:P, :].rearrange("p (a t) -> p a t", a=KT))
            for k in range(1, KT):
                ldeng.dma_start(out=wcat_sb[:, k], in_=wcat_r[:, k])
            xt1 = xin.tile([P, KT, P], DT, name="xt")
            ldeng.dma_start(out=xt1, in_=xT_d[P : 2 * P, :].rearrange("p (a t) -> p a t", a=KT))
            if has_bias:
                # row 0: column biases; row 1: ones (lhsT of rank-1 bias matmul)
                bias_sb = consts.tile([1, COLS], DT)
                ldeng.dma_start(out=bias_sb, in_=bias_d[0:1, :])
                ones_sb = consts.tile([1, P], DT)
                ldeng.dma_start(out=ones_sb, in_=bias_d[1:2, 0:P])
            leaf_sb = consts.tile([P, 2, O + 2], DT)
            ident = consts.tile([P, P], f32)
            masks.make_identity(nc, ident[:, :])
            if apply_affine:
                gamma_sb = consts.tile([P, O], f32)
                nc.gpsimd.dma_start(
                    out=gamma_sb, in_=gamma_d[:, :].to_broadcast((P, O))
                )
                beta_sb = consts.tile([P, O], f32)
                nc.gpsimd.dma_start(out=beta_sb, in_=beta_d[:, :].to_broadcast((P, O)))

            FB = max(BATCHES)
            batch_of = []
            for bi, bs in enumerate(BATCHES):
                batch_of += [(bi, q, bs) for q in range(bs)]
            batch_state = {}
            xt_queue = [xt0, xt1]
            ps1s = {}

            def phase1(j):
                """Prefetch x for tile j+2 and run tile j's mm1 on the PE."""
                if j + 2 < NTILES:
                    nxt = xin.tile([P, KT, P], DT, name="xt")
                    ldeng.dma_start(
                        out=nxt,
                        in_=xT_d[(j + 2) * P : (j + 3) * P, :].rearrange(
                            "p (a t) -> p a t", a=KT
                        ),
                    )
                    xt_queue.append(nxt)
                if j == 0:
                    # leaf weights are first needed by tile 0's mm2; issuing the
                    # load after xt2's keeps it off the startup critical path.
                    ldeng.dma_start(
                        out=leaf_sb,
                        in_=leaf_d[:, :].rearrange("(i p) o -> p i o", p=P),
                    )
                # ---- mm1: logits [128 tok, 240 dec | 16 gate] ----
                ps1 = psum1.tile([P, COLS], f32, name="ps1")
                for k in range(KT):
                    nc.tensor.matmul(
                        ps1, xt_queue[j][:, k], wcat_sb[:, k], start=(k == 0),
                        stop=(not has_bias and k == KT - 1),
                    )
                if has_bias:
                    # bias via rank-1 outer product: ones[tok] x biascat[cols]
                    nc.tensor.matmul(ps1, ones_sb, bias_sb, start=False, stop=True)
                ps1s[j] = ps1

            def phase2(j):
                _, jb, bsz = batch_of[j]
                ps1 = ps1s.pop(j)

                if jb == 0:
                    # per-batch stat accumulators: gate Z, sum(y) (=mean), sum(y^2)
                    gzall = statp.tile([P, FB], f32, name="gzall", tag="gzall")
                    syall = statp.tile([P, FB], f32, name="syall", tag="syall")
                    sqall = statp.tile([P, FB], f32, name="sqall", tag="sqall")
                    batch_state.clear()
                    batch_state.update(gzall=gzall, syall=syall, sqall=sqall,
                                       ps3s=[])
                else:
                    gzall = batch_state["gzall"]
                    syall = batch_state["syall"]
                    sqall = batch_state["sqall"]

                # ---- decisions + gate, both signs, contiguous ----
                # d2[:, 0, :] = sigmoid(logits)   (dec | gate-sigmoid)
                # d2[:, 1, :] = sigmoid(-logits)  (1-dec | 1-gate-sigmoid)
                d2 = d2p.tile([P, 2, COLS], f32, name="d2")
                nc.scalar.activation(out=d2[:, 0], in_=ps1, func=Act.Sigmoid)
                nc.scalar.activation(out=d2[:, 1], in_=ps1, func=Act.Sigmoid, scale=-1.0)
                # per-node decision view: dv[p, tree, node, sign]
                dv = d2[:, :, 0:NDEC].rearrange("p h (t n) -> p t n h", t=T)

                # ---- gate softmax without exp: e = s/(1-s), s = sigmoid ----
                # gate logits are x @ (0.02 randn) ~ N(0, 0.41): |logit| < ~6,
                # so no max-subtraction is needed in fp32.
                gr = gatep.tile([P, T], f32, name="gr")
                nc.vector.reciprocal(gr, d2[:, 1, NDEC:COLS])
                # ge = s * 1/(1-s); Z = sum(e) folded into the layernorm:
                # LN((y/Z - mu/Z)/sqrt(var/Z^2 + eps)) == (y-mu)/sqrt(var+eps*Z^2)
                ge = gatep.tile([P, T], f32, name="ge")
                nc.vector.tensor_mul(ge, d2[:, 0, NDEC:COLS], gr)
                nc.vector.reduce_sum(gzall[:, jb : jb + 1], ge, axis=X)

                # ---- gated leaf probabilities for all trees ----
                # Root level folded with the gate (32 cols), then cascade
                # 64 -> 128 -> 256; first two stages on DVE, last two on the
                # otherwise-idle GPSIMD engine.
                gg = mid.tile([P, T * 2], f32, name="gg")
                gev = ge.unsqueeze(-1).broadcast_to((P, T, 2))
                nc.vector.tensor_mul(
                    gg.rearrange("p (t c) -> p t c", t=T), dv[:, :, 0, :], gev
                )
                h01 = mid.tile([P, T * 4], f32, name="h01")
                ggv = (
                    gg.rearrange("p (t c) -> p t c", t=T)
                    .unsqueeze(-1)
                    .broadcast_to((P, T, 2, 2))
                )
                nc.gpsimd.tensor_mul(
                    h01.rearrange("p (t c r) -> p t c r", t=T, c=2), ggv,
                    dv[:, :, 1:3, :],
                )
                h012 = mid.tile([P, T * 8], f32, name="h012")
                h01v = (
                    h01.rearrange("p (t c) -> p t c", t=T)
                    .unsqueeze(-1)
                    .broadcast_to((P, T, 4, 2))
                )
                nc.gpsimd.tensor_mul(
                    h012.rearrange("p (t c r) -> p t c r", t=T, c=4), h01v,
                    dv[:, :, 3:7, :],
                )
                acc = mid.tile([P, T * NL], f32, name="acc")
                h012v = (
                    h012.rearrange("p (t c) -> p t c", t=T)
                    .unsqueeze(-1)
                    .broadcast_to((P, T, 8, 2))
                )
                nc.gpsimd.tensor_mul(
                    acc.rearrange("p (t c r) -> p t c r", t=T, c=8), h012v,
                    dv[:, :, 7:15, :],
                )

                # ---- transpose to (t,l)-major; col 256 takes the mean ----
                psT = psumT.tile([P, 2 * P + 2], f32, name="psT")
                nc.tensor.transpose(psT[:, 0:P], acc[:, 0:P], ident)
                nc.tensor.transpose(psT[:, P : 2 * P], acc[:, P : 2 * P], ident)
                wt = wtp.tile([P, 2, P], DT, name="wt")
                nc.vector.tensor_copy(
                    wt.rearrange("p i t -> p (i t)"), psT[:, 0 : 2 * P]
                )

                # ---- mm2: out_pre [128 tok, 512]; mean via rowsum column ----
                ps3 = psum3.tile([P, O], f32, name="ps3")
                nc.tensor.matmul(ps3, wt[:, 0], leaf_sb[:, 0, 0:O], start=True, stop=False)
                nc.tensor.matmul(ps3, wt[:, 1], leaf_sb[:, 1, 0:O], start=False, stop=True)
                nc.tensor.matmul(
                    psT[:, 2 * P : 2 * P + 2], wt[:, 0], leaf_sb[:, 0, O : O + 2],
                    start=True, stop=False,
                )
                nc.tensor.matmul(
                    psT[:, 2 * P : 2 * P + 2], wt[:, 1], leaf_sb[:, 1, O : O + 2],
                    start=False, stop=True,
                )
                nc.vector.tensor_copy(syall[:, jb : jb + 1], psT[:, 2 * P : 2 * P + 1])

                # ---- sum(y^2) in one ACT pass (Square shares the sigmoid
                # table set -- no table reload) ----
                scrap = scrapp.tile([P, O], f32, name="scrap")
                nc.scalar.activation(
                    out=scrap, in_=ps3, func=Act.Square,
                    accum_out=sqall[:, jb : jb + 1],
                )
                batch_state["ps3s"].append((j, ps3))

                if jb == bsz - 1:
                    # ---- batched LN finalize for the batch's tiles ----
                    fin_prio = tc.high_priority(offset=120)
                    fin_prio.__enter__()
                    mean_b = syall[:, 0:bsz]
                    gzb = gzall[:, 0:bsz]
                    # vt = E[y^2] - mean^2 + eps*Z^2
                    m2 = smalls.tile([P, FB], f32, name="m2")[:, 0:bsz]
                    nc.vector.tensor_mul(m2, mean_b, mean_b)
                    vt = smalls.tile([P, FB], f32, name="vt")[:, 0:bsz]
                    nc.vector.scalar_tensor_tensor(
                        out=vt, in0=sqall[:, 0:bsz], scalar=1.0 / O, in1=m2,
                        op0=Alu.mult, op1=Alu.subtract,
                    )
                    t2 = smalls.tile([P, FB], f32, name="t2")[:, 0:bsz]
                    nc.vector.scalar_tensor_tensor(
                        out=t2, in0=gzb, scalar=float(EPS), in1=gzb,
                        op0=Alu.mult, op1=Alu.mult,
                    )
                    nc.vector.tensor_add(vt, vt, t2)
                    # rstd via magic seed + Newton (keeps the ACT table set
                    # fixed at 'sigmoid' -- no 2.7us table reloads)
                    yt = smalls.tile([P, FB], f32, name="yt")[:, 0:bsz]
                    iv = smalls.tile([P, FB], i32, name="iv")[:, 0:bsz]
                    nc.vector.tensor_scalar(
                        out=iv, in0=vt.bitcast(i32), scalar1=1, scalar2=None,
                        op0=Alu.logical_shift_right,
                    )
                    nc.vector.tensor_scalar(
                        out=yt.bitcast(i32), in0=iv, scalar1=-1, scalar2=MAGIC,
                        op0=Alu.mult, op1=Alu.add,
                    )
                    for _ in range(2 if mm_dtype == "f32r" else 3):
                        aq = smalls.tile([P, FB], f32, name="aq", tag="aq")[:, 0:bsz]
                        nc.vector.tensor_mul(aq, yt, yt)
                        bq = smalls.tile([P, FB], f32, name="bq", tag="bq")[:, 0:bsz]
                        nc.vector.scalar_tensor_tensor(
                            out=bq, in0=vt, scalar=0.5, in1=aq,
                            op0=Alu.mult, op1=Alu.mult,
                        )
                        cq = smalls.tile([P, FB], f32, name="cq", tag="cq")[:, 0:bsz]
                        nc.vector.tensor_scalar(
                            out=cq, in0=bq, scalar1=-1.0, scalar2=1.5,
                            op0=Alu.mult, op1=Alu.add,
                        )
                        nc.vector.tensor_mul(yt, yt, cq)
                    nb = smalls.tile([P, FB], f32, name="nb")[:, 0:bsz]
                    nc.vector.scalar_tensor_tensor(
                        out=nb, in0=mean_b, scalar=-1.0, in1=yt,
                        op0=Alu.mult, op1=Alu.mult,
                    )
                    negmu = smalls.tile([P, FB], f32, name="negmu")[:, 0:bsz]
                    nc.vector.tensor_scalar(
                        out=negmu, in0=mean_b, scalar1=-1.0, scalar2=None,
                        op0=Alu.mult,
                    )

                    # ---- (x - mu) * rstd, one fused op per tile, alternating
                    # ACT / DVE so the batch finalize runs on both engines ----
                    for q, (jq, ps3q) in enumerate(batch_state["ps3s"]):
                        out_sb = outp.tile([P, O], ODT, name="out_sb")
                        if q % 2 == 0:
                            nc.scalar.activation(
                                out=out_sb, in_=ps3q, func=Act.Identity,
                                bias=nb[:, q : q + 1], scale=yt[:, q : q + 1],
                            )
                        else:
                            nc.vector.tensor_scalar(
                                out=out_sb, in0=ps3q,
                                scalar1=negmu[:, q : q + 1],
                                scalar2=yt[:, q : q + 1],
                                op0=Alu.add, op1=Alu.mult,
                            )
                        if apply_affine:
                            nc.vector.tensor_mul(out_sb, out_sb, gamma_sb)
                            nc.vector.tensor_add(out_sb, out_sb, beta_sb)
                        nc.sync.dma_start(
                            out=out_d[jq * P : (jq + 1) * P, :], in_=out_sb
                        )
                    fin_prio.__exit__(None, None, None)

            # software pipeline: the PE runs tile j+1's mm1 while tile j's
            # elementwise chain (sigmoid -> gate -> cascade) is in flight
            phase1(0)
            for j in range(NTILES):
                if j + 1 < NTILES:
                    phase1(j + 1)
                phase2(j)

    return nc


def _host_prep(x, decision_weights, decision_biases, leaf_outputs, gate_w, gate_b,
               node_temp_logits, ln_gamma, ln_beta):
    """Fold temperatures into weights/biases, transpose to matmul layouts,
    shard tokens across the 8 cores."""
    x = np.asarray(x, np.float32)
    temps = np.log1p(np.exp(np.asarray(node_temp_logits, np.float64) + 0.5413))
    temps = temps.astype(np.float32)  # TEMP == 1.0
    wd = (np.asarray(decision_weights, np.float32) / temps[..., None]).reshape(NDEC, D)
    wcat = np.concatenate([wd, np.asarray(gate_w, np.float32).T], axis=0)  # [256, D]
    wcatT = np.ascontiguousarray(wcat.T)  # [D, 256]
    biasrow = np.concatenate(
        [
            (np.asarray(decision_biases, np.float32) / temps).reshape(NDEC),
            np.asarray(gate_b, np.float32),
        ]
    )
    biascat = np.stack([biasrow, np.ones(COLS, np.float32)])  # [2, 256]
    leaf_flat = np.asarray(leaf_outputs, np.float32).reshape(T * NL, O)
    # column 512 = rowsum/O: the mean of each token's output lands straight
    # out of a rank-1 matmul against the (t,l)-major leaf probabilities
    # (column 513 is a dup pad: fp32r matmuls need an even free dim)
    rowsum = leaf_flat.sum(axis=1, keepdims=True) / O
    leaf2 = np.ascontiguousarray(np.concatenate([leaf_flat, rowsum, rowsum], axis=1))
    # per-core x: [8 tiles, 128 partitions(d mod), 8 ktiles, 128 tokens] so a
    # tile's DMA moves 4KiB-contiguous partition lines
    tokens = x.reshape(NCORES, TOK_PC, D)
    xTs = []
    for c in range(NCORES):
        xc = tokens[c].reshape(NTILES, P, KT, P)  # [j, tt, a, p]
        xc = np.ascontiguousarray(xc.transpose(0, 3, 2, 1))  # [j, p, a, tt]
        xTs.append(xc.reshape(NTILES * P, KT * P))
    gamma = np.asarray(ln_gamma, np.float32)
    beta = np.asarray(ln_beta, np.float32)
    affine = not (np.all(gamma == 1.0) and np.all(beta == 0.0))
    return xTs, wcatT, biascat, leaf2, gamma, beta, affine


_BUILT = {}


def _get_module(mm_dtype, apply_affine, has_bias):
    key = (mm_dtype, apply_affine, has_bias)
    if key not in _BUILT:
        nc = build(mm_dtype, apply_affine, has_bias)
        nc.compile()
        _BUILT[key] = nc
    return _BUILT[key]


def run_shards(in_maps, mm_dtype=MM_DTYPE, apply_affine=False, has_bias=True, trace=False):
    from concourse.bass_utils import run_bass_kernel_spmd

    nc = _get_module(mm_dtype, apply_affine, has_bias)
    return run_bass_kernel_spmd(nc, in_maps, list(range(NCORES)), trace=trace)


def make_in_maps(inputs):
    xTs, wcatT, biascat, leaf2, gamma, beta, affine = _host_prep(**inputs)
    has_bias = bool(np.any(biascat[0] != 0.0))
    in_maps = []
    for c in range(NCORES):
        m = {"xT": xTs[c], "wcatT": wcatT, "leaf2": leaf2}
        if has_bias:
            m["biascat"] = biascat
        if affine:
            m["gamma"] = gamma[None, :]
            m["beta"] = beta[None, :]
        in_maps.append(m)
    return in_maps, affine, has_bias


def kernel(**inputs) -> np.ndarray:
    B, S = inputs["x"].shape[:2]
    in_maps, affine, has_bias = make_in_maps(inputs)
    res = run_shards(in_maps, apply_affine=affine, has_bias=has_bias)
    out = np.concatenate(
        [np.asarray(res.results[c]["out"]).astype(np.float32) for c in range(NCORES)],
        axis=0,
    )
    return out.reshape(B, S, O)


# revision 13
# speedup vs baseline: 1.1505x; 1.1505x over previous
"""BatchedTreeForest (moe_routing) Trainium2 kernel.

Reference computation (B=4, S=2048, D=1024, O=512, T=16 trees, depth 4):
  logits  = einsum('bsd,tnd->bstn', x, W_dec) + b_dec          (15 internal nodes)
  dec     = sigmoid(logits / softplus(temp_logits + .5413))
  leafp   = prod over the 4 root->leaf path levels of (dec | 1-dec)
  per_tree= einsum('bstl,tlo->bsto', leafp, leaf_outputs)
  gate    = softmax(x @ gate_w + gate_b)
  out     = LayerNorm(einsum('bsto,bst->bso', per_tree, gate)) * gamma + beta

Mapping onto 8 NeuronCores: data-parallel over the 8192 tokens (1024/core),
tree/gate parameters replicated.  Per core, 8 tiles of 128 tokens,
software-pipelined so the PE runs tile j+1's mm1 while tile j's elementwise
chain (sigmoid -> gate -> leaf-product cascade) is in flight:
  mm1: xt(k-tiles) x [W_dec | gate_w] -> psum [128 tok, 256 cols]
  ACT sigmoid twice (scale=+1/-1) -> d2 [128, 2, 256]: contiguous [dec|gate]
       blocks for both signs (gate sigmoid/1-sigmoid come for free)
  gate softmax without exp: e = s/(1-s); Z folded into the LN epsilon
  leaf products: gate folded at the root level (32-col op on DVE), then a
       3-level cascade (64/128/256 cols) on the otherwise-idle GPSIMD
  PE transpose -> (t,l)-major; one DVE copy casts both halves to the matmul
       dtype; mm2 against leaf_outputs
  LayerNorm: DVE bn_stats/bn_aggr per tile; rsqrt(var + eps*Z^2) via magic
       seed + Newton, batched over (3,3,2) tiles (keeps the ACT table set
       fixed at 'sigmoid'); final (x-mu)*rstd fused, alternating ACT/DVE,
       reading the mm2 PSUM banks directly; stores in bf16 (upcast on host).

Host-side prep re-lays x out so every x DMA moves contiguous partition
lines, and casts x / weights to bf16 (TREE_IN_DTYPE=f32r to disable): the
kernel is DMA-bound at fp32, and bf16 halves the dominant x traffic while
keeping max rel err ~7e-3 vs the 2e-2 gate.
"""
import os
import sys

sys.path.insert(0, "/opt/trn_rl_repo")

import numpy as np

P = 128
D = 1024
T = 16
NI = 15
NL = 16
NDEC = T * NI  # 240
COLS = NDEC + T  # 256: decision logits | gate logits
O = 512
NCORES = 8
TOK_PC = 1024  # tokens per core
NTILES = TOK_PC // P
KT = D // P  # 8 contraction tiles
EPS = 1e-5
MAGIC = 0x5F3759DF

IN_DTYPE = os.environ.get("TREE_IN_DTYPE", "bf16")
OUT_DTYPE = os.environ.get("TREE_OUT_DTYPE", "bf16")
MM_DTYPE = IN_DTYPE  # back-compat alias for the test harness
BATCHES = (3, 3, 2)


def _np_in_dtype():
    if IN_DTYPE != "bf16":
        return np.float32
    try:
        import ml_dtypes

        return ml_dtypes.bfloat16
    except ImportError:
        import jax.numpy as jnp

        return jnp.bfloat16


def build(in_dtype: str = IN_DTYPE, apply_affine: bool = False, has_bias: bool = True,
          out_dtype: str = OUT_DTYPE):
    """Build the per-core Bass module.  Returns the Bacc object (uncompiled)."""
    import concourse.bacc as bacc
    import concourse.mybir as mybir
    from concourse import masks
    from concourse.tile import TileContext

    f32 = mybir.dt.float32
    i32 = mybir.dt.int32
    # bf16 default; float32r fallback (same bytes as fp32 on the wire, 4x the
    # fp32 matmul throughput)
    DT = mybir.dt.bfloat16 if in_dtype == "bf16" else mybir.dt.float32r
    ODT = mybir.dt.bfloat16 if out_dtype == "bf16" else f32
    Alu = mybir.AluOpType
    Act = mybir.ActivationFunctionType
    X = mybir.AxisListType.X

    nc = bacc.Bacc()
    xT_d = nc.declare_dram_parameter("xT", [NTILES * P, KT * P], DT, isOutput=False)
    wcat_d = nc.declare_dram_parameter("wcatT", [D, COLS], DT, isOutput=False)
    if has_bias:
        bias_d = nc.declare_dram_parameter("biascat", [2, COLS], DT, isOutput=False)
    leaf_d = nc.declare_dram_parameter("leaf2", [T * NL, O], DT, isOutput=False)
    if apply_affine:
        gamma_d = nc.declare_dram_parameter("gamma", [1, O], f32, isOutput=False)
        beta_d = nc.declare_dram_parameter("beta", [1, O], f32, isOutput=False)
    out_d = nc.declare_dram_parameter("out", [TOK_PC, O], ODT, isOutput=True)

    ldeng = nc.sync

    with TileContext(nc) as tc:
        with (
            tc.tile_pool(name="consts", bufs=1) as consts,
            tc.tile_pool(name="xin", bufs=4) as xin,
            tc.tile_pool(name="d2p", bufs=3) as d2p,
            tc.tile_pool(name="mid", bufs=3) as mid,
            tc.tile_pool(name="wtp", bufs=3) as wtp,
            tc.tile_pool(name="outp", bufs=NTILES) as outp,
            tc.tile_pool(name="gatep", bufs=4) as gatep,
            tc.tile_pool(name="statp", bufs=3) as statp,
            tc.tile_pool(name="smalls", bufs=4) as smalls,
            tc.tile_pool(name="psum1", bufs=2, space="PSUM") as psum1,
            tc.tile_pool(name="psumT", bufs=2, space="PSUM") as psumT,
            tc.tile_pool(name="psum3", bufs=4, space="PSUM") as psum3,
        ):
            # ---- constants + first x tiles, ordered so tile 0's matmuls can
            # start as soon as wcat k0 + xt0 have landed (HWDGE drains the SP
            # ring in issue order) ----
            wcat_sb = consts.tile([P, KT, COLS], DT)
            wcat_r = wcat_d[:, :].rearrange("(a p) c -> p a c", p=P)
            ldeng.dma_start(out=wcat_sb[:, 0], in_=wcat_r[:, 0])
            xt0 = xin.tile([P, KT, P], DT, name="xt")
            ldeng.dma_start(
                out=xt0, in_=xT_d[0:P, :].rearrange("p (a t) -> p a t", a=KT)
            )
            for k in range(1, KT):
                ldeng.dma_start(out=wcat_sb[:, k], in_=wcat_r[:, k])
            xt1 = xin.tile([P, KT, P], DT, name="xt")
            ldeng.dma_start(
                out=xt1, in_=xT_d[P : 2 * P, :].rearrange("p (a t) -> p a t", a=KT)
            )
            if has_bias:
                # row 0: column biases; row 1: ones (lhsT of rank-1 bias matmul)
                bias_sb = consts.tile([1, COLS], DT)
                ldeng.dma_start(out=bias_sb, in_=bias_d[0:1, :])
                ones_sb = consts.tile([1, P], DT)
                ldeng.dma_start(out=ones_sb, in_=bias_d[1:2, 0:P])
            leaf_sb = consts.tile([P, 2, O], DT)
            ident = consts.tile([P, P], f32)
            masks.make_identity(nc, ident[:, :])
            if apply_affine:
                gamma_sb = consts.tile([P, O], f32)
                nc.gpsimd.dma_start(
                    out=gamma_sb, in_=gamma_d[:, :].to_broadcast((P, O))
                )
                beta_sb = consts.tile([P, O], f32)
                nc.gpsimd.dma_start(out=beta_sb, in_=beta_d[:, :].to_broadcast((P, O)))

            FB = max(BATCHES)
            batch_of = []
            for bi, bs in enumerate(BATCHES):
                batch_of += [(bi, q, bs) for q in range(bs)]
            batch_state = {}
            xt_queue = [xt0, xt1]
            ps1s = {}

            def phase1(j):
                """Prefetch x for tile j+2 and run tile j's mm1 on the PE."""
                if j + 2 < NTILES:
                    nxt = xin.tile([P, KT, P], DT, name="xt")
                    ldeng.dma_start(
                        out=nxt,
                        in_=xT_d[(j + 2) * P : (j + 3) * P, :].rearrange(
                            "p (a t) -> p a t", a=KT
                        ),
                    )
                    xt_queue.append(nxt)
                if j == 0:
                    # leaf weights are first needed by tile 0's mm2; issuing the
                    # load after xt2's keeps it off the startup critical path.
                    ldeng.dma_start(
                        out=leaf_sb,
                        in_=leaf_d[:, :].rearrange("(i p) o -> p i o", p=P),
                    )
                # ---- mm1: logits [128 tok, 240 dec | 16 gate] ----
                ps1 = psum1.tile([P, COLS], f32, name="ps1")
                for k in range(KT):
                    nc.tensor.matmul(
                        ps1, xt_queue[j][:, k], wcat_sb[:, k], start=(k == 0),
                        stop=(not has_bias and k == KT - 1),
                    )
                if has_bias:
                    # bias via rank-1 outer product: ones[tok] x biascat[cols]
                    nc.tensor.matmul(ps1, ones_sb, bias_sb, start=False, stop=True)
                ps1s[j] = ps1

            def phase2(j):
                _, jb, bsz = batch_of[j]
                ps1 = ps1s.pop(j)

                if jb == 0:
                    # per-batch stat accumulators
                    mvall = statp.tile([P, FB, 2], f32, name="mvall", tag="mvall")
                    gzall = statp.tile([P, FB], f32, name="gzall", tag="gzall")
                    batch_state.clear()
                    batch_state.update(mvall=mvall, gzall=gzall, ps3s=[])
                else:
                    mvall = batch_state["mvall"]
                    gzall = batch_state["gzall"]

                # ---- decisions + gate, both signs, contiguous ----
                # d2[:, 0, :] = sigmoid(logits)   (dec | gate-sigmoid)
                # d2[:, 1, :] = sigmoid(-logits)  (1-dec | 1-gate-sigmoid)
                d2 = d2p.tile([P, 2, COLS], f32, name="d2")
                nc.scalar.activation(out=d2[:, 0], in_=ps1, func=Act.Sigmoid)
                nc.scalar.activation(out=d2[:, 1], in_=ps1, func=Act.Sigmoid, scale=-1.0)
                # per-node decision view: dv[p, tree, node, sign]
                dv = d2[:, :, 0:NDEC].rearrange("p h (t n) -> p t n h", t=T)

                # ---- gate softmax without exp: e = s/(1-s), s = sigmoid ----
                # gate logits are x @ (0.02 randn) ~ N(0, 0.41): |logit| < ~6,
                # so no max-subtraction is needed in fp32.
                gr = gatep.tile([P, T], f32, name="gr")
                nc.vector.reciprocal(gr, d2[:, 1, NDEC:COLS])
                ge = gatep.tile([P, T], f32, name="ge")
                nc.vector.tensor_mul(ge, d2[:, 0, NDEC:COLS], gr)
                # Z = sum(e) folded into the layernorm instead of dividing:
                # LN((y/Z - mu/Z)/sqrt(var/Z^2 + eps)) == (y-mu)/sqrt(var+eps*Z^2)
                nc.vector.reduce_sum(gzall[:, jb : jb + 1], ge, axis=X)

                # ---- gated leaf probabilities for all trees ----
                # Root level folded with the gate (32 cols, DVE), then the
                # cascade 64 -> 128 -> 256 on GPSIMD.
                gg = mid.tile([P, T * 2], f32, name="gg")
                gev = ge.unsqueeze(-1).broadcast_to((P, T, 2))
                nc.vector.tensor_mul(
                    gg.rearrange("p (t c) -> p t c", t=T), dv[:, :, 0, :], gev
                )
                h01 = mid.tile([P, T * 4], f32, name="h01")
                ggv = (
                    gg.rearrange("p (t c) -> p t c", t=T)
                    .unsqueeze(-1)
                    .broadcast_to((P, T, 2, 2))
                )
                nc.gpsimd.tensor_mul(
                    h01.rearrange("p (t c r) -> p t c r", t=T, c=2), ggv,
                    dv[:, :, 1:3, :],
                )
                h012 = mid.tile([P, T * 8], f32, name="h012")
                h01v = (
                    h01.rearrange("p (t c) -> p t c", t=T)
                    .unsqueeze(-1)
                    .broadcast_to((P, T, 4, 2))
                )
                nc.gpsimd.tensor_mul(
                    h012.rearrange("p (t c r) -> p t c r", t=T, c=4), h01v,
                    dv[:, :, 3:7, :],
                )
                acc = mid.tile([P, T * NL], f32, name="acc")
                h012v = (
                    h012.rearrange("p (t c) -> p t c", t=T)
                    .unsqueeze(-1)
                    .broadcast_to((P, T, 8, 2))
                )
                nc.gpsimd.tensor_mul(
                    acc.rearrange("p (t c r) -> p t c r", t=T, c=8), h012v,
                    dv[:, :, 7:15, :],
                )

                # ---- transpose to (t,l)-major for mm2 ----
                psT = psumT.tile([P, 2 * P], f32, name="psT")
                nc.tensor.transpose(psT[:, 0:P], acc[:, 0:P], ident)
                nc.tensor.transpose(psT[:, P : 2 * P], acc[:, P : 2 * P], ident)
                wt = wtp.tile([P, 2, P], DT, name="wt")
                nc.vector.tensor_copy(
                    wt.rearrange("p i t -> p (i t)"), psT[:, 0 : 2 * P]
                )

                # ---- mm2: out_pre [128 tok, 512] ----
                ps3 = psum3.tile([P, O], f32, name="ps3")
                nc.tensor.matmul(ps3, wt[:, 0], leaf_sb[:, 0], start=True, stop=False)
                nc.tensor.matmul(ps3, wt[:, 1], leaf_sb[:, 1], start=False, stop=True)

                # ---- layernorm stats ----
                st6 = smalls.tile([P, 6], f32, name="st6")
                nc.vector.bn_stats(st6, ps3)
                nc.vector.bn_aggr(mvall[:, jb, :], st6)
                batch_state["ps3s"].append((j, ps3))

                if jb == bsz - 1:
                    # ---- batched LN finalize for the batch's tiles ----
                    fin_prio = tc.high_priority(offset=120)
                    fin_prio.__enter__()
                    # vt = var_pre + eps*Z^2 ; rstd via magic seed + Newton on
                    # [128, bsz] (no ACT table switch away from sigmoid).
                    var4 = mvall[:, 0:bsz, 1]
                    mean4 = mvall[:, 0:bsz, 0]
                    gzb = gzall[:, 0:bsz]
                    vt = smalls.tile([P, FB], f32, name="vt")[:, 0:bsz]
                    nc.vector.scalar_tensor_tensor(
                        out=vt, in0=gzb, scalar=float(EPS), in1=gzb,
                        op0=Alu.mult, op1=Alu.mult,
                    )
                    nc.vector.tensor_add(vt, vt, var4)
                    yt = smalls.tile([P, FB], f32, name="yt")[:, 0:bsz]
                    iv = smalls.tile([P, FB], i32, name="iv")[:, 0:bsz]
                    nc.vector.tensor_scalar(
                        out=iv, in0=vt.bitcast(i32), scalar1=1, scalar2=None,
                        op0=Alu.logical_shift_right,
                    )
                    nc.vector.tensor_scalar(
                        out=yt.bitcast(i32), in0=iv, scalar1=-1, scalar2=MAGIC,
                        op0=Alu.mult, op1=Alu.add,
                    )
                    # one Newton step leaves ~2e-3 rel on rstd -- inside the
                    # bf16 error budget (two steps in f32r mode)
                    for _ in range(1 if in_dtype == "bf16" else 2):
                        aq = smalls.tile([P, FB], f32, name="aq", tag="aq")[:, 0:bsz]
                        nc.vector.tensor_mul(aq, yt, yt)
                        bq = smalls.tile([P, FB], f32, name="bq", tag="bq")[:, 0:bsz]
                        nc.vector.scalar_tensor_tensor(
                            out=bq, in0=vt, scalar=0.5, in1=aq,
                            op0=Alu.mult, op1=Alu.mult,
                        )
                        cq = smalls.tile([P, FB], f32, name="cq", tag="cq")[:, 0:bsz]
                        nc.vector.tensor_scalar(
                            out=cq, in0=bq, scalar1=-1.0, scalar2=1.5,
                            op0=Alu.mult, op1=Alu.add,
                        )
                        nc.vector.tensor_mul(yt, yt, cq)
                    nb = smalls.tile([P, FB], f32, name="nb")[:, 0:bsz]
                    nc.vector.scalar_tensor_tensor(
                        out=nb, in0=mean4, scalar=-1.0, in1=yt,
                        op0=Alu.mult, op1=Alu.mult,
                    )
                    negmu = smalls.tile([P, FB], f32, name="negmu")[:, 0:bsz]
                    nc.vector.tensor_scalar(
                        out=negmu, in0=mean4, scalar1=-1.0, scalar2=None,
                        op0=Alu.mult,
                    )

                    # ---- (x - mu) * rstd, one fused op per tile, alternating
                    # ACT / DVE so the batch finalize runs on both engines ----
                    for q, (jq, ps3q) in enumerate(batch_state["ps3s"]):
                        out_sb = outp.tile([P, O], ODT, name="out_sb")
                        if q % 2 == 0:
                            nc.scalar.activation(
                                out=out_sb, in_=ps3q, func=Act.Identity,
                                bias=nb[:, q : q + 1], scale=yt[:, q : q + 1],
                            )
                        else:
                            nc.vector.tensor_scalar(
                                out=out_sb, in0=ps3q,
                                scalar1=negmu[:, q : q + 1],
                                scalar2=yt[:, q : q + 1],
                                op0=Alu.add, op1=Alu.mult,
                            )
                        if apply_affine:
                            nc.vector.tensor_mul(out_sb, out_sb, gamma_sb)
                            nc.vector.tensor_add(out_sb, out_sb, beta_sb)
                        nc.sync.dma_start(
                            out=out_d[jq * P : (jq + 1) * P, :], in_=out_sb
                        )
                    fin_prio.__exit__(None, None, None)

            # software pipeline: the PE runs tile j+1's mm1 while tile j's
            # elementwise chain (sigmoid -> gate -> cascade) is in flight
            phase1(0)
            for j in range(NTILES):
                if j + 1 < NTILES:
                    phase1(j + 1)
                phase2(j)

    return nc


def _host_prep(x, decision_weights, decision_biases, leaf_outputs, gate_w, gate_b,
               node_temp_logits, ln_gamma, ln_beta):
    """Fold temperatures into weights/biases, transpose to matmul layouts,
    shard tokens across the 8 cores."""
    ndt = _np_in_dtype()
    x = np.asarray(x, np.float32)
    temps = np.log1p(np.exp(np.asarray(node_temp_logits, np.float64) + 0.5413))
    temps = temps.astype(np.float32)  # TEMP == 1.0
    wd = (np.asarray(decision_weights, np.float32) / temps[..., None]).reshape(NDEC, D)
    wcat = np.concatenate([wd, np.asarray(gate_w, np.float32).T], axis=0)  # [256, D]
    wcatT = np.ascontiguousarray(wcat.T).astype(ndt)  # [D, 256]
    biasrow = np.concatenate(
        [
            (np.asarray(decision_biases, np.float32) / temps).reshape(NDEC),
            np.asarray(gate_b, np.float32),
        ]
    )
    biascat = np.stack([biasrow, np.ones(COLS, np.float32)]).astype(ndt)  # [2, 256]
    leaf2 = np.ascontiguousarray(
        np.asarray(leaf_outputs, np.float32).reshape(T * NL, O)
    ).astype(ndt)
    # per-core x: [8 tiles, 128 partitions(d mod), 8 ktiles, 128 tokens] so a
    # tile's DMA moves contiguous partition lines
    tokens = x.reshape(NCORES, TOK_PC, D)
    xTs = []
    for c in range(NCORES):
        xc = tokens[c].reshape(NTILES, P, KT, P)  # [j, tt, a, p]
        xc = np.ascontiguousarray(xc.transpose(0, 3, 2, 1))  # [j, p, a, tt]
        xTs.append(xc.reshape(NTILES * P, KT * P).astype(ndt))
    gamma = np.asarray(ln_gamma, np.float32)
    beta = np.asarray(ln_beta, np.float32)
    affine = not (np.all(gamma == 1.0) and np.all(beta == 0.0))
    return xTs, wcatT, biascat, leaf2, gamma, beta, affine


_BUILT = {}


def _get_module(in_dtype, apply_affine, has_bias):
    key = (in_dtype, apply_affine, has_bias)
    if key not in _BUILT:
        nc = build(in_dtype, apply_affine, has_bias)
        nc.compile()
        _BUILT[key] = nc
    return _BUILT[key]


def run_shards(in_maps, in_dtype=IN_DTYPE, apply_affine=False, has_bias=True, trace=False):
    from concourse.bass_utils import run_bass_kernel_spmd

    nc = _get_module(in_dtype, apply_affine, has_bias)
    return run_bass_kernel_spmd(nc, in_maps, list(range(NCORES)), trace=trace)


def make_in_maps(inputs):
    xTs, wcatT, biascat, leaf2, gamma, beta, affine = _host_prep(**inputs)
    has_bias = bool(np.any(np.asarray(biascat[0], np.float32) != 0.0))
    in_maps = []
    for c in range(NCORES):
        m = {"xT": xTs[c], "wcatT": wcatT, "leaf2": leaf2}
        if has_bias:
            m["biascat"] = biascat
        if affine:
            m["gamma"] = gamma[None, :]
            m["beta"] = beta[None, :]
        in_maps.append(m)
    return in_maps, affine, has_bias


def kernel(**inputs) -> np.ndarray:
    B, S = inputs["x"].shape[:2]
    in_maps, affine, has_bias = make_in_maps(inputs)
    res = run_shards(in_maps, apply_affine=affine, has_bias=has_bias)
    out = np.concatenate(
        [np.asarray(res.results[c]["out"]).astype(np.float32) for c in range(NCORES)],
        axis=0,
    )
    return out.reshape(B, S, O)


# revision 16
# speedup vs baseline: 1.3117x; 1.1400x over previous
"""BatchedTreeForest (moe_routing) Trainium2 kernel.

Reference computation (B=4, S=2048, D=1024, O=512, T=16 trees, depth 4):
  logits  = einsum('bsd,tnd->bstn', x, W_dec) + b_dec          (15 internal nodes)
  dec     = sigmoid(logits / softplus(temp_logits + .5413))
  leafp   = prod over the 4 root->leaf path levels of (dec | 1-dec)
  per_tree= einsum('bstl,tlo->bsto', leafp, leaf_outputs)
  gate    = softmax(x @ gate_w + gate_b)
  out     = LayerNorm(einsum('bsto,bst->bso', per_tree, gate)) * gamma + beta

Mapping onto 8 NeuronCores: data-parallel over the 8192 tokens (1024/core),
tree/gate parameters replicated.  Per core, 8 tiles of 128 tokens,
software-pipelined so the PE runs tile j+1's mm1 while tile j's elementwise
chain (sigmoid -> gate -> leaf-product cascade) is in flight:
  mm1: xt(k-tiles) x [W_dec | gate_w] -> psum [128 tok, 256 cols]
  ACT sigmoid twice (scale=+1/-1) -> d2 [128, 2, 256]: contiguous [dec|gate]
       blocks for both signs (gate sigmoid/1-sigmoid come for free)
  gate softmax without exp: e = s/(1-s); Z folded into the LN epsilon
  leaf products: gate folded at the root level (32-col op on DVE), then a
       3-level cascade (64/128/256 cols) on the otherwise-idle GPSIMD
  PE transpose -> (t,l)-major; one DVE copy casts both halves to the matmul
       dtype; mm2 against leaf_outputs
  LayerNorm: DVE bn_stats/bn_aggr per tile; rsqrt(var + eps*Z^2) via magic
       seed + Newton, batched over (3,3,2) tiles (keeps the ACT table set
       fixed at 'sigmoid'); final (x-mu)*rstd fused, alternating ACT/DVE,
       reading the mm2 PSUM banks directly; stores in bf16 (upcast on host).

Host-side prep re-lays x out so every x DMA moves contiguous partition
lines, and casts x / weights to bf16 (TREE_IN_DTYPE=f32r to disable): the
kernel is DMA-bound at fp32, and bf16 halves the dominant x traffic while
keeping max rel err ~7e-3 vs the 2e-2 gate.
"""
import os
import sys

sys.path.insert(0, "/opt/trn_rl_repo")

import numpy as np

P = 128
D = 1024
T = 16
NI = 15
NL = 16
NDEC = T * NI  # 240
COLS = NDEC + T  # 256: decision logits | gate logits
O = 512
NCORES = 8
TOK_PC = 1024  # tokens per core
NTILES = TOK_PC // P
KT = D // P  # 8 contraction tiles
EPS = 1e-5
MAGIC = 0x5F3759DF

IN_DTYPE = os.environ.get("TREE_IN_DTYPE", "bf16")
OUT_DTYPE = os.environ.get("TREE_OUT_DTYPE", "bf16")
MM_DTYPE = IN_DTYPE  # back-compat alias for the test harness
BATCHES = (3, 3, 2)


def _np_in_dtype():
    if IN_DTYPE != "bf16":
        return np.float32
    try:
        import ml_dtypes

        return ml_dtypes.bfloat16
    except ImportError:
        import jax.numpy as jnp

        return jnp.bfloat16


def build(in_dtype: str = IN_DTYPE, apply_affine: bool = False, has_bias: bool = True,
          out_dtype: str = OUT_DTYPE):
    """Build the per-core Bass module.  Returns the Bacc object (uncompiled)."""
    import concourse.bacc as bacc
    import concourse.mybir as mybir
    from concourse import masks
    from concourse.tile import TileContext

    f32 = mybir.dt.float32
    i32 = mybir.dt.int32
    # bf16 default; float32r fallback (same bytes as fp32 on the wire, 4x the
    # fp32 matmul throughput)
    DT = mybir.dt.bfloat16 if in_dtype == "bf16" else mybir.dt.float32r
    ODT = mybir.dt.bfloat16 if out_dtype == "bf16" else f32
    Alu = mybir.AluOpType
    Act = mybir.ActivationFunctionType
    X = mybir.AxisListType.X

    nc = bacc.Bacc()
    # x: [4 tile-pairs x 128 partitions, pair-interleaved k/token cols] so one
    # DMA moves two tiles with 4KiB-contiguous partition lines (bf16)
    xT_d = nc.declare_dram_parameter("xT", [(NTILES // 2) * P, 2 * KT * P], DT,
                                     isOutput=False)
    # wcat/leaf host-packed to [128, ...] so each loads in a single DMA with
    # multi-KiB partition lines
    wcat_d = nc.declare_dram_parameter("wcatT", [P, KT * COLS], DT, isOutput=False)
    if has_bias:
        bias_d = nc.declare_dram_parameter("biascat", [2, COLS], DT, isOutput=False)
    leaf_d = nc.declare_dram_parameter("leaf2", [P, 2 * O], DT, isOutput=False)
    if apply_affine:
        gamma_d = nc.declare_dram_parameter("gamma", [1, O], f32, isOutput=False)
        beta_d = nc.declare_dram_parameter("beta", [1, O], f32, isOutput=False)
    # out: [4 tile-pairs x 128 rows, pair-interleaved halves] (host un-permutes)
    out_d = nc.declare_dram_parameter("out", [(NTILES // 2) * P, 2 * O], ODT,
                                      isOutput=True)

    ldeng = nc.sync

    with TileContext(nc) as tc:
        with (
            tc.tile_pool(name="consts", bufs=1) as consts,
            tc.tile_pool(name="xin", bufs=4) as xin,
            tc.tile_pool(name="d2p", bufs=3) as d2p,
            tc.tile_pool(name="mid", bufs=3) as mid,
            tc.tile_pool(name="wtp", bufs=3) as wtp,
            tc.tile_pool(name="outp", bufs=NTILES // 2) as outp,
            tc.tile_pool(name="gatep", bufs=4) as gatep,
            tc.tile_pool(name="statp", bufs=3) as statp,
            tc.tile_pool(name="smalls", bufs=4) as smalls,
            tc.tile_pool(name="psum1", bufs=2, space="PSUM") as psum1,
            tc.tile_pool(name="psumT", bufs=2, space="PSUM") as psumT,
            tc.tile_pool(name="psum3", bufs=4, space="PSUM") as psum3,
        ):
            # ---- constants + first x tiles, ordered so tile 0's matmuls can
            # start as soon as wcat k0 + xt0 have landed (HWDGE drains the SP
            # ring in issue order) ----
            wcat_sb = consts.tile([P, KT, COLS], DT)
            wcat_r = wcat_d[:, :].rearrange("p (a c) -> p a c", a=KT)
            ldeng.dma_start(out=wcat_sb[:, 0 : KT // 2], in_=wcat_r[:, 0 : KT // 2])
            xt_pairs = []

            def load_xpair(pr):
                xp = xin.tile([P, 2, KT, P], DT, name="xp")
                ldeng.dma_start(
                    out=xp,
                    in_=xT_d[pr * P : (pr + 1) * P, :].rearrange(
                        "p (i a t) -> p i a t", i=2, a=KT
                    ),
                )
                xt_pairs.append(xp)

            load_xpair(0)
            ldeng.dma_start(out=wcat_sb[:, KT // 2 :], in_=wcat_r[:, KT // 2 :])
            if has_bias:
                # row 0: column biases; row 1: ones (lhsT of rank-1 bias matmul)
                bias_sb = consts.tile([1, COLS], DT)
                ldeng.dma_start(out=bias_sb, in_=bias_d[0:1, :])
                ones_sb = consts.tile([1, P], DT)
                ldeng.dma_start(out=ones_sb, in_=bias_d[1:2, 0:P])
            leaf_sb = consts.tile([P, 2, O], DT)
            ident = consts.tile([P, P], f32)
            masks.make_identity(nc, ident[:, :])
            # dummy sigmoid: pulls the 1.3us ACT table load into the x-load
            # phase instead of tile 0's critical path
            warm = consts.tile([1, 2], f32)
            nc.scalar.activation(out=warm, in_=ident[0:1, 0:2], func=Act.Sigmoid)
            if apply_affine:
                gamma_sb = consts.tile([P, O], f32)
                nc.gpsimd.dma_start(
                    out=gamma_sb, in_=gamma_d[:, :].to_broadcast((P, O))
                )
                beta_sb = consts.tile([P, O], f32)
                nc.gpsimd.dma_start(out=beta_sb, in_=beta_d[:, :].to_broadcast((P, O)))

            FB = max(BATCHES)
            batch_of = []
            for bi, bs in enumerate(BATCHES):
                batch_of += [(bi, q, bs) for q in range(bs)]
            batch_state = {}
            ps1s = {}
            out_pairs = {}

            def phase1(j):
                """Prefetch x for tiles j+2/j+3 and run tile j's mm1 on the PE."""
                if j % 2 == 0 and j + 2 < NTILES:
                    load_xpair(j // 2 + 1)
                if j == 0:
                    # leaf weights are first needed by tile 0's mm2; issuing the
                    # load after xpair1's keeps it off the startup critical path.
                    ldeng.dma_start(
                        out=leaf_sb,
                        in_=leaf_d[:, :].rearrange("p (i o) -> p i o", i=2),
                    )
                # ---- mm1: logits [128 tok, 240 dec | 16 gate] ----
                xt = xt_pairs[j // 2][:, j % 2]
                ps1 = psum1.tile([P, COLS], f32, name="ps1")
                for k in range(KT):
                    nc.tensor.matmul(
                        ps1, xt[:, k], wcat_sb[:, k], start=(k == 0),
                        stop=(not has_bias and k == KT - 1),
                    )
                if has_bias:
                    # bias via rank-1 outer product: ones[tok] x biascat[cols]
                    nc.tensor.matmul(ps1, ones_sb, bias_sb, start=False, stop=True)
                ps1s[j] = ps1

            def phase2(j):
                _, jb, bsz = batch_of[j]
                ps1 = ps1s.pop(j)

                if jb == 0:
                    # per-batch stat accumulators
                    mvall = statp.tile([P, FB, 2], f32, name="mvall", tag="mvall")
                    gzall = statp.tile([P, FB], f32, name="gzall", tag="gzall")
                    batch_state.clear()
                    batch_state.update(mvall=mvall, gzall=gzall, ps3s=[])
                else:
                    mvall = batch_state["mvall"]
                    gzall = batch_state["gzall"]

                # ---- decisions + gate, both signs, contiguous ----
                # d2[:, 0, :] = sigmoid(logits)   (dec | gate-sigmoid)
                # d2[:, 1, :] = sigmoid(-logits)  (1-dec | 1-gate-sigmoid)
                d2 = d2p.tile([P, 2, COLS], f32, name="d2")
                nc.scalar.activation(out=d2[:, 0], in_=ps1, func=Act.Sigmoid)
                nc.scalar.activation(out=d2[:, 1], in_=ps1, func=Act.Sigmoid, scale=-1.0)
                # per-node decision view: dv[p, tree, node, sign]
                dv = d2[:, :, 0:NDEC].rearrange("p h (t n) -> p t n h", t=T)

                # ---- gate softmax without exp: e = s/(1-s), s = sigmoid ----
                # gate logits are x @ (0.02 randn) ~ N(0, 0.41): |logit| < ~6,
                # so no max-subtraction is needed in fp32.
                gr = gatep.tile([P, T], f32, name="gr")
                nc.vector.reciprocal(gr, d2[:, 1, NDEC:COLS])
                ge = gatep.tile([P, T], f32, name="ge")
                nc.vector.tensor_mul(ge, d2[:, 0, NDEC:COLS], gr)
                # Z = sum(e) folded into the layernorm instead of dividing:
                # LN((y/Z - mu/Z)/sqrt(var/Z^2 + eps)) == (y-mu)/sqrt(var+eps*Z^2)
                nc.vector.reduce_sum(gzall[:, jb : jb + 1], ge, axis=X)

                # ---- gated leaf probabilities for all trees ----
                # Root level folded with the gate (32 cols, DVE), then the
                # cascade 64 -> 128 -> 256 on GPSIMD.
                gg = mid.tile([P, T * 2], f32, name="gg")
                gev = ge.unsqueeze(-1).broadcast_to((P, T, 2))
                nc.gpsimd.tensor_mul(
                    gg.rearrange("p (t c) -> p t c", t=T), dv[:, :, 0, :], gev
                )
                h01 = mid.tile([P, T * 4], f32, name="h01")
                ggv = (
                    gg.rearrange("p (t c) -> p t c", t=T)
                    .unsqueeze(-1)
                    .broadcast_to((P, T, 2, 2))
                )
                nc.gpsimd.tensor_mul(
                    h01.rearrange("p (t c r) -> p t c r", t=T, c=2), ggv,
                    dv[:, :, 1:3, :],
                )
                h012 = mid.tile([P, T * 8], f32, name="h012")
                h01v = (
                    h01.rearrange("p (t c) -> p t c", t=T)
                    .unsqueeze(-1)
                    .broadcast_to((P, T, 4, 2))
                )
                nc.gpsimd.tensor_mul(
                    h012.rearrange("p (t c r) -> p t c r", t=T, c=4), h01v,
                    dv[:, :, 3:7, :],
                )
                acc = mid.tile([P, T * NL], f32, name="acc")
                h012v = (
                    h012.rearrange("p (t c) -> p t c", t=T)
                    .unsqueeze(-1)
                    .broadcast_to((P, T, 8, 2))
                )
                nc.gpsimd.tensor_mul(
                    acc.rearrange("p (t c r) -> p t c r", t=T, c=8), h012v,
                    dv[:, :, 7:15, :],
                )

                # ---- transpose to (t,l)-major for mm2 ----
                psT = psumT.tile([P, 2 * P], f32, name="psT")
                nc.tensor.transpose(psT[:, 0:P], acc[:, 0:P], ident)
                nc.tensor.transpose(psT[:, P : 2 * P], acc[:, P : 2 * P], ident)
                wt = wtp.tile([P, 2, P], DT, name="wt")
                nc.scalar.copy(wt.rearrange("p i t -> p (i t)"), psT[:, 0 : 2 * P])

                # ---- mm2: out_pre [128 tok, 512] ----
                ps3 = psum3.tile([P, O], f32, name="ps3")
                nc.tensor.matmul(ps3, wt[:, 0], leaf_sb[:, 0], start=True, stop=False)
                nc.tensor.matmul(ps3, wt[:, 1], leaf_sb[:, 1], start=False, stop=True)

                # ---- layernorm stats ----
                st6 = smalls.tile([P, 6], f32, name="st6")
                nc.vector.bn_stats(st6, ps3)
                nc.vector.bn_aggr(mvall[:, jb, :], st6)
                batch_state["ps3s"].append((j, ps3))

                if jb == bsz - 1:
                    # ---- batched LN finalize for the batch's tiles ----
                    fin_prio = tc.high_priority(offset=120)
                    fin_prio.__enter__()
                    # vt = var_pre + eps*Z^2 ; rstd via magic seed + Newton on
                    # [128, bsz] (no ACT table switch away from sigmoid).
                    var4 = mvall[:, 0:bsz, 1]
                    mean4 = mvall[:, 0:bsz, 0]
                    gzb = gzall[:, 0:bsz]
                    vt = smalls.tile([P, FB], f32, name="vt")[:, 0:bsz]
                    nc.vector.scalar_tensor_tensor(
                        out=vt, in0=gzb, scalar=float(EPS), in1=gzb,
                        op0=Alu.mult, op1=Alu.mult,
                    )
                    nc.vector.tensor_add(vt, vt, var4)
                    yt = smalls.tile([P, FB], f32, name="yt")[:, 0:bsz]
                    iv = smalls.tile([P, FB], i32, name="iv")[:, 0:bsz]
                    nc.vector.tensor_scalar(
                        out=iv, in0=vt.bitcast(i32), scalar1=1, scalar2=None,
                        op0=Alu.logical_shift_right,
                    )
                    nc.vector.tensor_scalar(
                        out=yt.bitcast(i32), in0=iv, scalar1=-1, scalar2=MAGIC,
                        op0=Alu.mult, op1=Alu.add,
                    )
                    # one Newton step leaves ~2e-3 rel on rstd -- inside the
                    # bf16 error budget (two steps in f32r mode)
                    for _ in range(1 if in_dtype == "bf16" else 2):
                        aq = smalls.tile([P, FB], f32, name="aq", tag="aq")[:, 0:bsz]
                        nc.vector.tensor_mul(aq, yt, yt)
                        bq = smalls.tile([P, FB], f32, name="bq", tag="bq")[:, 0:bsz]
                        nc.vector.scalar_tensor_tensor(
                            out=bq, in0=vt, scalar=0.5, in1=aq,
                            op0=Alu.mult, op1=Alu.mult,
                        )
                        cq = smalls.tile([P, FB], f32, name="cq", tag="cq")[:, 0:bsz]
                        nc.vector.tensor_scalar(
                            out=cq, in0=bq, scalar1=-1.0, scalar2=1.5,
                            op0=Alu.mult, op1=Alu.add,
                        )
                        nc.vector.tensor_mul(yt, yt, cq)
                    nb = smalls.tile([P, FB], f32, name="nb")[:, 0:bsz]
                    nc.vector.scalar_tensor_tensor(
                        out=nb, in0=mean4, scalar=-1.0, in1=yt,
                        op0=Alu.mult, op1=Alu.mult,
                    )
                    negmu = smalls.tile([P, FB], f32, name="negmu")[:, 0:bsz]
                    nc.vector.tensor_scalar(
                        out=negmu, in0=mean4, scalar1=-1.0, scalar2=None,
                        op0=Alu.mult,
                    )

                    # ---- (x - mu) * rstd, one fused op per tile, alternating
                    # ACT / DVE so the batch finalize runs on both engines;
                    # tile pairs share one SBUF tile -> one 2KiB-line store ----
                    for q, (jq, ps3q) in enumerate(batch_state["ps3s"]):
                        pr, half = jq // 2, jq % 2
                        if half == 0:
                            out_sb = outp.tile([P, 2, O], ODT, name="out_sb",
                                               tag=f"out{pr}")
                            out_pairs[pr] = out_sb
                        else:
                            out_sb = out_pairs[pr]
                        if q % 2 == 0:
                            nc.scalar.activation(
                                out=out_sb[:, half], in_=ps3q, func=Act.Identity,
                                bias=nb[:, q : q + 1], scale=yt[:, q : q + 1],
                            )
                        else:
                            nc.vector.tensor_scalar(
                                out=out_sb[:, half], in0=ps3q,
                                scalar1=negmu[:, q : q + 1],
                                scalar2=yt[:, q : q + 1],
                                op0=Alu.add, op1=Alu.mult,
                            )
                        if apply_affine:
                            nc.vector.tensor_mul(out_sb[:, half], out_sb[:, half], gamma_sb)
                            nc.vector.tensor_add(out_sb[:, half], out_sb[:, half], beta_sb)
                        if half == 1:
                            nc.sync.dma_start(
                                out=out_d[pr * P : (pr + 1) * P, :],
                                in_=out_sb.rearrange("p i o -> p (i o)"),
                            )
                    fin_prio.__exit__(None, None, None)

            # software pipeline: the PE runs tile j+1's mm1 while tile j's
            # elementwise chain (sigmoid -> gate -> cascade) is in flight
            phase1(0)
            for j in range(NTILES):
                if j + 1 < NTILES:
                    phase1(j + 1)
                phase2(j)

    return nc


def _host_prep(x, decision_weights, decision_biases, leaf_outputs, gate_w, gate_b,
               node_temp_logits, ln_gamma, ln_beta):
    """Fold temperatures into weights/biases, transpose to matmul layouts,
    shard tokens across the 8 cores."""
    ndt = _np_in_dtype()
    x = np.asarray(x, np.float32)
    temps = np.log1p(np.exp(np.asarray(node_temp_logits, np.float64) + 0.5413))
    temps = temps.astype(np.float32)  # TEMP == 1.0
    wd = (np.asarray(decision_weights, np.float32) / temps[..., None]).reshape(NDEC, D)
    wcat = np.concatenate([wd, np.asarray(gate_w, np.float32).T], axis=0)  # [256, D]
    # packed [128, KT*256]: row p holds k-tile-major columns for d = a*128+p
    wcatT = np.ascontiguousarray(
        wcat.T.reshape(KT, P, COLS).transpose(1, 0, 2).reshape(P, KT * COLS)
    ).astype(ndt)
    biasrow = np.concatenate(
        [
            (np.asarray(decision_biases, np.float32) / temps).reshape(NDEC),
            np.asarray(gate_b, np.float32),
        ]
    )
    biascat = np.stack([biasrow, np.ones(COLS, np.float32)]).astype(ndt)  # [2, 256]
    # packed [128, 2*512]: row p holds both (t,l) halves for rows i*128+p
    leaf2 = np.ascontiguousarray(
        np.asarray(leaf_outputs, np.float32)
        .reshape(2, P, O)
        .transpose(1, 0, 2)
        .reshape(P, 2 * O)
    ).astype(ndt)
    # per-core x: [8 tiles, 128 partitions(d mod), 8 ktiles, 128 tokens] so a
    # tile's DMA moves contiguous partition lines
    tokens = x.reshape(NCORES, TOK_PC, D)
    xTs = []
    for c in range(NCORES):
        # [pair, p, i(tile of pair), a(ktile), tt] -> 4KiB bf16 partition lines
        xc = tokens[c].reshape(NTILES // 2, 2, P, KT, P)  # [pr, i, tt, a, p]
        xc = np.ascontiguousarray(xc.transpose(0, 4, 1, 3, 2))  # [pr, p, i, a, tt]
        xTs.append(xc.reshape((NTILES // 2) * P, 2 * KT * P).astype(ndt))
    gamma = np.asarray(ln_gamma, np.float32)
    beta = np.asarray(ln_beta, np.float32)
    affine = not (np.all(gamma == 1.0) and np.all(beta == 0.0))
    return xTs, wcatT, biascat, leaf2, gamma, beta, affine


_BUILT = {}


def _get_module(in_dtype, apply_affine, has_bias):
    key = (in_dtype, apply_affine, has_bias)
    if key not in _BUILT:
        nc = build(in_dtype, apply_affine, has_bias)
        nc.compile()
        _BUILT[key] = nc
    return _BUILT[key]


def run_shards(in_maps, in_dtype=IN_DTYPE, apply_affine=False, has_bias=True, trace=False):
    from concourse.bass_utils import run_bass_kernel_spmd

    nc = _get_module(in_dtype, apply_affine, has_bias)
    return run_bass_kernel_spmd(nc, in_maps, list(range(NCORES)), trace=trace)


def make_in_maps(inputs):
    xTs, wcatT, biascat, leaf2, gamma, beta, affine = _host_prep(**inputs)
    has_bias = bool(np.any(np.asarray(biascat[0], np.float32) != 0.0))
    in_maps = []
    for c in range(NCORES):
        m = {"xT": xTs[c], "wcatT": wcatT, "leaf2": leaf2}
        if has_bias:
            m["biascat"] = biascat
        if affine:
            m["gamma"] = gamma[None, :]
            m["beta"] = beta[None, :]
        in_maps.append(m)
    return in_maps, affine, has_bias


def kernel(**inputs) -> np.ndarray:
    B, S = inputs["x"].shape[:2]
    in_maps, affine, has_bias = make_in_maps(inputs)
    res = run_shards(in_maps, apply_affine=affine, has_bias=has_bias)
    outs = []
    for c in range(NCORES):
        buf = np.asarray(res.results[c]["out"]).astype(np.float32)
        # [pair*128, 2*512] -> tokens pair*256 + i*128 + p
        buf = buf.reshape(NTILES // 2, P, 2, O).transpose(0, 2, 1, 3)
        outs.append(buf.reshape(TOK_PC, O))
    return np.concatenate(outs, axis=0).reshape(B, S, O)
